# revision 39
# baseline (speedup 1.0000x reference)
"""Trainium2 Bass kernel for banded (sparse) decoder attention.

Reference (per batch b):
    kvp = kv @ Wkv -> k, v (8 heads x 64);  qh = q @ Wq
    S = qh k^T * hd^-0.5, band |i-j|<=w, softmax;  x = P v
    out = x @ Wproj + bproj

Sharding: 8 cores = batch(4) x seq-half(2); each core does 1024 rows of
one batch with a +-w kv halo. All matmuls bf16 with fp32 PSUM accum.

The end-to-end wall time is dominated by the axon host<->device tunnel
(~70 MB/s up, ~33 MB/s down, ~70 ms fixed RPC round-trip per jit call),
so the design minimizes per-call bytes and RPCs:

  - activations kv/q are int8-quantized per token on host (rel err
    ~1.1e-2 end to end vs the 2e-2 gate) and uploaded token-major as a
    single [2*SEQ+2w, C] int8 shard per core, streamed with async
    per-device device_put while the next batch quantizes on host;
  - weights (Wkv/Wq/Wproj/bias/band-mask), packed 128-partition
    feature-major, are uploaded once and kept device-resident;
  - one cached jit(shard_map) over the Bass NEFF (mirroring
    bass2jax.run_bass_via_pjrt) runs all 8 cores; donated output
    buffers are recycled call-to-call so no zero-buffers are uploaded;
  - the output is int8 row-quantized on device (scale = rowmax/127);
    the f32 row scales ride in the same flat int8 output tensor,
    bitcast into its tail; output shards are fetched concurrently
    (blocking pulls only pipeline when issued in parallel) and each is
    dequantized to f32 on the main thread as it arrives, so downloads
    of early cores overlap uploads of later ones end to end.

Device pipeline per core:
  - int8 [128, C] token tiles: dequant (DVE per-partition scale) to
    bf16, PE-transpose into feature-major kvT / qT
  - kT (feature-major), v (token-major), qhT projections via PE
  - per 128-query tile, per 2-head group: S matmuls into PSUM;
    multiplicative band mask (DVE) on exp(S); v carries a ones column
    so P^T @ v also yields softmax row-sums; 1/rowsum applied per head
    during the x PSUM->SBUF copy; PE-transpose x; output projection +
    bias; int8 row-quantize; DMA out.
"""

import numpy as np
import ml_dtypes

B, N, C, H = 4, 2048, 512, 8
HD = C // H  # 64
NCORES = 8
SEQ = N // 2  # rows per core
SCALE = HD ** -0.5
PB = 128
PWP = SEQ + PB  # padded kv rows per core
HG = 2          # heads per processing group


def _band_w(epoch: int):
    if epoch >= 60:
        return None
    if epoch < 22:
        return 4
    if epoch < 32:
        return 6
    if epoch < 42:
        return 8
    return 10


def _build_nc(w: int):
    import concourse.mybir as mybir
    import concourse.tile as tile
    from concourse import bacc
    from concourse.masks import make_identity

    f32 = mybir.dt.float32
    bf16 = mybir.dt.bfloat16
    i8 = mybir.dt.int8
    AF = mybir.ActivationFunctionType

    W2 = 2 * w
    WIN = PB + W2
    NQT = SEQ // PB
    CC = C // PB
    NVT = PWP // PB
    NG = H // HG

    KVROWS = SEQ + 2 * w      # uploaded kv rows per core (halo, no tile pad)
    T2 = KVROWS + SEQ         # total uploaded token rows per core
    NTILES = (T2 + PB - 1) // PB

    nc = bacc.Bacc(None, target_bir_lowering=False)
    # per-call activations: token-major int8 + per-token dequant scales;
    # rows [0:KVROWS] are kv (with halo), rows [KVROWS:] are q
    acts_d = nc.declare_dram_parameter("acts", [T2, C], i8, isOutput=False)
    ascale_d = nc.declare_dram_parameter("ascale", [PB, NTILES], f32, isOutput=False)
    # resident weights
    wkv_d = nc.declare_dram_parameter("wkv", [PB, CC * 2 * C], bf16, isOutput=False)
    wq_d = nc.declare_dram_parameter("wq", [PB, CC * C], bf16, isOutput=False)
    wp_d = nc.declare_dram_parameter("wp", [PB, CC * C], bf16, isOutput=False)
    bias_d = nc.declare_dram_parameter("bias_b", [PB, C], f32, isOutput=False)
    mask_d = nc.declare_dram_parameter(
        "mask", [PB, NQT * 2 * PB], bf16, isOutput=False
    )
    # flat int8 output: [SEQ*C] quantized rows, then the f32 per-query-row
    # scales bitcast to int8 as [PB, NQT*4]
    out_d = nc.declare_dram_parameter(
        "out", [SEQ * C + PB * NQT * 4], i8, isOutput=True
    )

    with tile.TileContext(nc) as tc:
        with (
            tc.sbuf_pool(name="const", bufs=1) as cpool,
            tc.sbuf_pool(name="work", bufs=3) as wpool,
            tc.psum_pool(name="psum", bufs=1) as ppool,
        ):
            # ---- persistent SBUF ----
            wq_s = cpool.tile([PB, CC, C], bf16)
            nc.sync.dma_start(wq_s, wq_d[:, :])
            wkv_s = cpool.tile([PB, CC, 2 * C], bf16)
            nc.sync.dma_start(wkv_s, wkv_d[:, :])
            wp_s = cpool.tile([PB, CC, C], bf16)
            nc.sync.dma_start(wp_s, wp_d[:, :])
            bias_s = cpool.tile([PB, C], f32)
            nc.sync.dma_start(bias_s, bias_d[:, :])
            mask_s = cpool.tile([PB, NQT, 2 * PB], bf16)
            nc.sync.dma_start(mask_s, mask_d[:, :])
            ident = cpool.tile([PB, PB], bf16)
            make_identity(nc, ident)

            # ---- dequant + on-device transpose into feature-major ----
            acts_s = cpool.tile([PB, NTILES, C], i8)
            for t in range(NTILES):
                lo, hi = t * PB, min((t + 1) * PB, T2)
                nc.sync.dma_start(acts_s[0 : hi - lo, t, :], acts_d[lo:hi, :])
            ascale_s = cpool.tile([PB, NTILES], f32)
            nc.sync.dma_start(ascale_s, ascale_d[:, :])

            kvT = cpool.tile([PB, CC, PWP], bf16)
            # key columns beyond the halo are never valid (mask gates them)
            # but must be finite zeros for the projections
            nc.vector.memset(kvT[:, :, KVROWS:PWP], 0.0)
            qT = cpool.tile([PB, CC, SEQ], bf16)
            for t in range(NTILES):
                lo, hi = t * PB, min((t + 1) * PB, T2)
                deq = wpool.tile([PB, C], bf16, tag="deq", bufs=2)
                nc.vector.tensor_scalar_mul(
                    deq, acts_s[:, t, :], ascale_s[:, t : t + 1]
                )
                trp = ppool.tile([PB, C], bf16, tag="big", bufs=2)
                for cc in range(CC):
                    nc.tensor.transpose(
                        trp[:, cc * PB : (cc + 1) * PB],
                        deq[:, cc * PB : (cc + 1) * PB],
                        ident,
                    )
                src3 = trp.rearrange("p (c q) -> p c q", c=CC)
                # route transposed columns: kv rows -> kvT, q rows -> qT
                segs = []
                if lo < KVROWS:
                    ln = min(hi, KVROWS) - lo
                    segs.append((kvT, lo, 0, ln))
                    if hi > KVROWS:
                        segs.append((qT, 0, ln, hi - KVROWS))
                else:
                    segs.append((qT, lo - KVROWS, 0, hi - lo))
                for dstbuf, dcol, scol, ln in segs:
                    nc.any.tensor_copy(
                        dstbuf[:, :, dcol : dcol + ln],
                        src3[:, :, scol : scol + ln],
                    )

            kT = cpool.tile([PB, CC, PWP], bf16)
            qhT = cpool.tile([PB, CC, SEQ], bf16)
            # v with an appended ones column per head: mm2 then yields
            # softmax row-sums for free in output column HD
            v_s = cpool.tile([PB, NVT, H, HD + 1], bf16)
            nc.vector.memset(v_s[:, :, :, HD], 1.0)

            def proj_T(dst, src, wsb, wofs, seqlen):
                segs = []
                s0 = 0
                while s0 < seqlen:
                    segs.append((s0, min(512, seqlen - s0)))
                    s0 += 512
                for co in range(CC):
                    for s0, sl in segs:
                        ps = ppool.tile([PB, 512], f32, tag="big", bufs=2)
                        for ci in range(CC):
                            nc.tensor.matmul(
                                ps[:, :sl],
                                wsb[:, ci, wofs + co * PB : wofs + (co + 1) * PB],
                                src[:, ci, s0 : s0 + sl],
                                start=(ci == 0),
                                stop=(ci == CC - 1),
                            )
                        nc.any.tensor_copy(dst[:, co, s0 : s0 + sl], ps[:, :sl])

            proj_T(qhT, qT, wq_s, 0, SEQ)
            proj_T(kT, kvT, wkv_s, 0, PWP)
            for i in range(NVT):
                ps = ppool.tile([PB, C], f32, tag="big", bufs=2)
                for ci in range(CC):
                    nc.tensor.matmul(
                        ps,
                        kvT[:, ci, i * PB : (i + 1) * PB],
                        wkv_s[:, ci, C : 2 * C],
                        start=(ci == 0),
                        stop=(ci == CC - 1),
                    )
                nc.any.tensor_copy(
                    v_s[:, i, :, :HD],
                    ps.rearrange("p (h d) -> p h d", d=HD),
                )

            # ---- attention + output projection per 128-query tile ----
            oscale_sb = cpool.tile([PB, NQT], f32)
            HH = H // 2  # heads per x psum half
            for t in range(NQT):
                x_half = [
                    ppool.tile([PB, HH, HD + 1], f32, tag="x", bufs=2, name=f"xh{t}_{i}")
                    for i in range(2)
                ]
                rinv = wpool.tile([PB, H], f32, tag="rinv", bufs=2)
                x_sb = wpool.tile([PB, C], bf16, tag="x_sb", bufs=2)
                for g in range(NG):
                    for hh in range(HG):
                        h = g * HG + hh
                        hc, hp = h // 2, (h % 2) * HD
                        # S^T against key tiles t and t+1 (band always fits):
                        # [key, chunk*query] layout, so P^T feeds mm2 directly
                        st = ppool.tile(
                            [PB, 256], f32, tag="s", bufs=4, name=f"st{t}_{h}"
                        )
                        for c in range(2):
                            nc.tensor.matmul(
                                st[:, c * PB : (c + 1) * PB],
                                kT[
                                    hp : hp + HD,
                                    hc,
                                    (t + c) * PB : (t + c + 1) * PB,
                                ],
                                qhT[hp : hp + HD, hc, t * PB : (t + 1) * PB],
                                start=True,
                                stop=True,
                            )
                        est = wpool.tile([PB, 256], bf16, tag="est", bufs=4)
                        nc.scalar.activation(est, st, AF.Exp, scale=SCALE)
                        nc.vector.tensor_mul(est, est, mask_s[:, t, :])
                        xp = x_half[h // HH]
                        for c in range(2):
                            nc.tensor.matmul(
                                xp[:, h % HH, :],
                                est[:, c * PB : (c + 1) * PB],
                                v_s[:, t + c, h, :],
                                start=(c == 0),
                                stop=(c == 1),
                            )
                    if (g * HG + HG) % HH == 0:
                        # heads for this x half done: 1/rowsum, normalize
                        half = (g * HG + HG) // HH - 1
                        xp = x_half[half]
                        nc.vector.reciprocal(
                            rinv[:, half * HH : (half + 1) * HH],
                            xp[:, :, HD],
                        )
                        for hh2 in range(HH):
                            h2 = half * HH + hh2
                            dst = x_sb[:, h2 * HD : (h2 + 1) * HD]
                            if hh2 % 2 == 0:
                                nc.vector.tensor_scalar_mul(
                                    dst, xp[:, hh2, :HD], rinv[:, h2 : h2 + 1]
                                )
                            else:
                                nc.scalar.activation(
                                    dst,
                                    xp[:, hh2, :HD],
                                    AF.Copy,
                                    scale=rinv[:, h2 : h2 + 1],
                                )
                xt_ps = ppool.tile([PB, C], bf16, tag="big", bufs=2)
                for ccI in range(CC):
                    nc.tensor.transpose(
                        xt_ps[:, ccI * PB : (ccI + 1) * PB],
                        x_sb[:, ccI * PB : (ccI + 1) * PB],
                        ident,
                    )
                xt_sb = wpool.tile([PB, C], bf16, tag="xt_sb")
                nc.any.tensor_copy(xt_sb, xt_ps)
                o_ps = ppool.tile([PB, C], f32, tag="big", bufs=2)
                for ci in range(CC):
                    nc.tensor.matmul(
                        o_ps,
                        xt_sb[:, ci * PB : (ci + 1) * PB],
                        wp_s[:, ci, :],
                        start=(ci == 0),
                        stop=(ci == CC - 1),
                    )
                out_sb = wpool.tile([PB, C], f32, tag="out_sb")
                nc.vector.tensor_add(out_sb, o_ps, bias_s)
                # int8 row-quantization: scale = rowmax/127
                absb = wpool.tile([PB, C], f32, tag="absb", bufs=2)
                nc.scalar.activation(absb, out_sb, AF.Abs)
                mx8 = wpool.tile([PB, 8], f32, tag="mx8", bufs=2)
                nc.vector.max(mx8, absb)
                rin = wpool.tile([PB, 2], f32, tag="rin", bufs=2)
                nc.vector.reciprocal(rin[:, 0:1], mx8[:, 0:1])
                nc.vector.tensor_scalar_mul(rin[:, 1:2], rin[:, 0:1], 127.0)
                oq = wpool.tile([PB, C], i8, tag="oq", bufs=2)
                nc.scalar.activation(oq, out_sb, AF.Copy, scale=rin[:, 1:2])
                nc.sync.dma_start(
                    out_d[t * PB * C : (t + 1) * PB * C].rearrange(
                        "(p c) -> p c", c=C
                    ),
                    oq,
                )
                nc.vector.tensor_scalar_mul(
                    oscale_sb[:, t : t + 1], mx8[:, 0:1], 1.0 / 127.0
                )
            nc.sync.dma_start(
                out_d[SEQ * C : SEQ * C + PB * NQT * 4].rearrange(
                    "(p j) -> p j", j=NQT * 4
                ),
                oscale_sb.bitcast(i8),
            )

    nc.compile()
    return nc


_NC_CACHE = {}
_RUNNER_CACHE = {}
LAST_RESULTS = None


def _get_nc(w: int):
    if w not in _NC_CACHE:
        _NC_CACHE[w] = _build_nc(w)
    return _NC_CACHE[w]


def _numpy_reference(kv, q, Wkv, Wq, Wproj, bproj, epoch):
    # dense fallback (epoch >= 60)
    b, n, c = kv.shape
    hd = c // H
    kvp = (kv @ Wkv).reshape(b, n, 2, H, hd)
    k = kvp[:, :, 0].transpose(0, 2, 1, 3)
    v = kvp[:, :, 1].transpose(0, 2, 1, 3)
    qh = (q @ Wq).reshape(b, n, H, hd).transpose(0, 2, 1, 3)
    attn = np.einsum("bhnd,bhmd->bhnm", qh, k) * (hd ** -0.5)
    w = _band_w(int(epoch))
    if w is not None:
        idx = np.arange(n)
        mask = np.abs(idx[:, None] - idx[None, :]) <= w
        attn = np.where(mask[None, None], attn, np.float32(-1e9))
    attn = attn - attn.max(axis=-1, keepdims=True)
    attn = np.exp(attn)
    attn /= attn.sum(axis=-1, keepdims=True)
    x = np.einsum("bhnm,bhmd->bhnd", attn, v)
    x = x.transpose(0, 2, 1, 3).reshape(b, n, c)
    return (x @ Wproj + bproj).astype(np.float32)


def _chunkW(wmat):
    """[C, M] -> [128, CC*M]: out[p, cc*M+m] = w[cc*128+p, m]"""
    M = wmat.shape[1]
    return np.ascontiguousarray(
        wmat.reshape(-1, PB, M).transpose(1, 0, 2).reshape(PB, -1)
    )


def _pack_weights(Wkv, Wq, Wproj, bproj, w):
    """Weight-class tensors: uploaded once, kept device-resident."""
    bf = ml_dtypes.bfloat16
    W2, NQT = 2 * w, SEQ // PB
    shared = {
        "wkv": _chunkW(Wkv).astype(bf),
        "wq": _chunkW(Wq).astype(bf),
        "wp": _chunkW(Wproj).astype(bf),
        "bias_b": np.broadcast_to(bproj, (PB, C)).copy().astype(np.float32),
    }

    # band mask in S^T-chunk coords [t, k, c, q]
    t_idx = np.arange(NQT)[:, None, None, None]
    k_idx = np.arange(PB)[None, :, None, None]
    c_idx = np.arange(2)[None, None, :, None]
    q_idx = np.arange(PB)[None, None, None, :]
    band2 = (q_idx <= c_idx * PB + k_idx) & (c_idx * PB + k_idx <= q_idx + W2)

    masks = []
    for core in range(NCORES):
        b, half = divmod(core, 2)
        r0 = half * SEQ
        # S^T chunk mask: entry [k, t, c*128+q] gates key 128(t+c)+k
        # (padded coords) against query 128t+q
        kg = r0 + (t_idx + c_idx) * PB + k_idx - w
        valid = band2 & (kg >= 0) & (kg < N)
        masks.append(valid.astype(np.float32).transpose(1, 0, 2, 3).reshape(PB, -1))
    sharded = {"mask": np.ascontiguousarray(np.concatenate(masks, 0)).astype(bf)}
    return shared, sharded


def _quant_into(a, dst, tmp):
    """int8-quantize [R, C] rows with per-row scales directly into dst."""
    s = np.abs(a).max(axis=1)
    r = 127.0 / np.maximum(s, 1e-30)
    t = tmp[: a.shape[0]]
    np.multiply(a, r[:, None], out=t)
    np.rint(t, out=t)
    np.copyto(dst, t, casting="unsafe")
    return s * (1.0 / 127.0)


def _pack_and_put(kv, q, w, runner):
    """Quantize + pack per-core [T2, C] int8 shards, streaming each to its
    device as soon as it's built so transfers overlap packing."""
    import jax

    KVROWS = SEQ + 2 * w
    T2 = KVROWS + SEQ
    NTILES = (T2 + PB - 1) // PB
    tmp = np.empty((KVROWS, C), np.float32)
    arrs = []
    scales_all = np.full((NCORES, NTILES * PB), 1.0 / 127.0, np.float32)
    for core in range(NCORES):
        b, half = divmod(core, 2)
        r0 = half * SEQ
        lo, hi = max(0, r0 - w), min(N, r0 + SEQ + w)
        d0 = lo - (r0 - w)
        shard = np.zeros((T2, C), np.int8)
        scales_all[core, d0 : d0 + hi - lo] = _quant_into(
            kv[b, lo:hi], shard[d0 : d0 + hi - lo], tmp
        )
        scales_all[core, KVROWS : KVROWS + SEQ] = _quant_into(
            q[b, r0 : r0 + SEQ], shard[KVROWS:], tmp
        )
        arrs.append(jax.device_put(shard, runner.devices[core]))
    acts = jax.make_array_from_single_device_arrays(
        (NCORES * T2, C), runner.sh_core, arrs
    )
    ascale = np.ascontiguousarray(
        scales_all.reshape(NCORES, NTILES, PB).transpose(0, 2, 1)
    ).reshape(NCORES * PB, NTILES)
    return acts, ascale


class _Runner:
    """Cached jit(shard_map) over the Bass NEFF with device-resident weights
    and donated-output recycling (mirrors bass2jax.run_bass_via_pjrt)."""

    PER_CALL = ("acts", "ascale")
    SHARDED_WEIGHTS = ("mask",)

    def __init__(self, nc):
        import jax
        import concourse.mybir as mybir
        from concourse import bass2jax
        from jax.sharding import Mesh, PartitionSpec, NamedSharding

        bass2jax.install_neuronx_cc_hook()
        try:
            from jax.experimental.shard_map import shard_map
        except ImportError:
            import functools

            shard_map = functools.partial(jax.shard_map)

        self.jax = jax
        self.nc = nc
        devices = jax.devices()[:NCORES]
        assert len(devices) == NCORES
        self.devices = devices
        self.mesh = Mesh(np.asarray(devices), ("core",))
        P = PartitionSpec
        self.sh_core = NamedSharding(self.mesh, P("core"))
        self.sh_repl = NamedSharding(self.mesh, P())

        partition_name = (
            nc.partition_id_tensor.name if nc.partition_id_tensor else None
        )
        in_names, out_names, out_avals, zero_outs = [], [], [], []
        in_specs = []
        for alloc in nc.m.functions[0].allocations:
            if not isinstance(alloc, mybir.MemoryLocationSet):
                continue
            name = alloc.memorylocations[0].name
            if alloc.kind == "ExternalInput":
                if name != partition_name:
                    in_names.append(name)
                    in_specs.append(
                        P("core")
                        if (name in self.PER_CALL or name in self.SHARDED_WEIGHTS)
                        else P()
                    )
            elif alloc.kind == "ExternalOutput":
                assert alloc.tensor_shape is not None and alloc.dtype is not None
                out_names.append(name)
                shape = tuple(alloc.tensor_shape)
                dtype = mybir.dt.np(alloc.dtype)
                out_avals.append(jax.core.ShapedArray(shape, dtype))
                zero_outs.append(
                    np.zeros((NCORES * shape[0], *shape[1:]), dtype)
                )
        self.in_names = in_names
        self.out_names = out_names
        n_params = len(in_names)
        all_in_names = tuple(in_names) + tuple(out_names)
        if partition_name is not None:
            all_in_names = all_in_names + (partition_name,)
        donate = tuple(range(n_params, n_params + len(out_names)))
        in_specs = tuple(in_specs) + (P("core"),) * len(out_names)
        out_specs = (P("core"),) * len(out_names)
        out_avals_t = tuple(out_avals)

        def _body(*args):
            operands = list(args)
            if partition_name is not None:
                operands.append(bass2jax.partition_id_tensor())
            outs = bass2jax._bass_exec_p.bind(
                *operands,
                out_avals=out_avals_t,
                in_names=all_in_names,
                out_names=tuple(out_names),
                lowering_input_output_aliases=(),
                sim_require_finite=True,
                sim_require_nnan=True,
                nc=nc,
            )
            return tuple(outs)

        self.fn = jax.jit(
            shard_map(
                _body,
                mesh=self.mesh,
                in_specs=in_specs,
                out_specs=out_specs,
                check_rep=False,
            ),
            donate_argnums=donate,
            keep_unused=True,
        )
        # donated output buffers, recycled call-to-call (kernel writes every
        # element of out, so stale contents are harmless)
        self.zero_outs = zero_outs
        self.recycle = [jax.device_put(z, self.sh_core) for z in zero_outs]
        self.weights_raw = None
        self.weights_dev = None

    def ensure_weights(self, Wkv, Wq, Wproj, bproj, w):
        raw = (Wkv, Wq, Wproj, bproj)
        if self.weights_raw is not None and all(
            a is b or np.array_equal(a, b)
            for a, b in zip(self.weights_raw, raw)
        ):
            return
        shared, sharded = _pack_weights(Wkv, Wq, Wproj, bproj, w)
        dev = {}
        for name, arr in shared.items():
            dev[name] = self.jax.device_put(arr, self.sh_repl)
        for name, arr in sharded.items():
            dev[name] = self.jax.device_put(arr, self.sh_core)
        for v in dev.values():
            v.block_until_ready()
        self.weights_dev = dev
        self.weights_raw = tuple(np.array(a, copy=True) for a in raw)

    def run(self, per_call):
        """Dispatch; returns the on-device output arrays (not fetched)."""
        args = [
            per_call[name] if name in self.PER_CALL else self.weights_dev[name]
            for name in self.in_names
        ]
        try:
            outs = self.fn(*args, *self.recycle)
        except Exception:
            # the donated recycle buffers may have been consumed; restore
            # fresh ones so a retry by the caller can still work
            self.recycle = [
                self.jax.device_put(z, self.sh_core) for z in self.zero_outs
            ]
            raise
        self.recycle = list(outs)
        return outs


def _get_runner(w: int):
    if w not in _RUNNER_CACHE:
        _RUNNER_CACHE[w] = _Runner(_get_nc(w))
    return _RUNNER_CACHE[w]


def kernel(**inputs):
    kv = np.asarray(inputs["kv"], np.float32)
    q = np.asarray(inputs["q"], np.float32)
    Wkv = np.asarray(inputs["Wkv"], np.float32)
    Wq = np.asarray(inputs["Wq"], np.float32)
    Wproj = np.asarray(inputs["Wproj"], np.float32)
    bproj = np.asarray(inputs["bproj"], np.float32)
    epoch = int(np.asarray(inputs["epoch"]))

    w = _band_w(epoch)
    if w is None:
        return _numpy_reference(kv, q, Wkv, Wq, Wproj, bproj, epoch)

    runner = _get_runner(w)
    runner.ensure_weights(Wkv, Wq, Wproj, bproj, w)
    acts, ascale = _pack_and_put(kv, q, w, runner)
    outs = runner.run({"acts": acts, "ascale": ascale})

    # fetch shards concurrently (round trips pipeline only when issued in
    # parallel), dequantizing each on the main thread as it completes
    from concurrent.futures import ThreadPoolExecutor, as_completed

    NQT = SEQ // PB
    PER = SEQ * C + PB * NQT * 4
    out = np.empty((NCORES, SEQ, C), np.float32)
    with ThreadPoolExecutor(NCORES) as ex:
        futs = {
            ex.submit(lambda s=s: np.asarray(s.data)): s.index[0].start // PER
            for s in outs[0].addressable_shards
        }
        for f in as_completed(futs):
            core = futs[f]
            arr = f.result()
            rows = arr[SEQ * C :].view(np.float32).reshape(PB, NQT).T.ravel()
            np.multiply(
                arr[: SEQ * C].reshape(SEQ, C), rows[:, None], out=out[core]
            )
    return out.reshape(B, N, C)



# revision 41
# speedup vs baseline: 1.4008x; 1.4008x over previous
"""Trainium2 Bass kernel for banded (sparse) decoder attention.

Reference (per batch b):
    kvp = kv @ Wkv -> k, v (8 heads x 64);  qh = q @ Wq
    S = qh k^T * hd^-0.5, band |i-j|<=w, softmax;  x = P v
    out = x @ Wproj + bproj

Sharding: 8 cores = batch(4) x seq-half(2); each core does 1024 rows of
one batch with a +-w kv halo. All matmuls bf16 with fp32 PSUM accum.

The end-to-end wall time is dominated by the axon host<->device tunnel
(~70 MB/s up, ~33 MB/s down, ~70 ms fixed RPC round-trip per jit call),
so the design minimizes per-call bytes and RPCs:

  - activations kv/q are int8-quantized per token on host (rel err
    ~1.1e-2 end to end vs the 2e-2 gate) and uploaded token-major as a
    single [2*SEQ+2w, C] int8 shard per core, streamed with async
    per-device device_put while the next batch quantizes on host;
  - weights (Wkv/Wq/Wproj/bias/band-mask), packed 128-partition
    feature-major, are uploaded once and kept device-resident;
  - one cached jit(shard_map) over the Bass NEFF (mirroring
    bass2jax.run_bass_via_pjrt) runs all 8 cores; donated output
    buffers are recycled call-to-call so no zero-buffers are uploaded;
  - the output is int8 row-quantized on device (scale = rowmax/127);
    the f32 row scales ride in the same flat int8 output tensor,
    bitcast into its tail; output shards are fetched concurrently
    (blocking pulls only pipeline when issued in parallel) and each is
    dequantized to f32 on the main thread as it arrives, so downloads
    of early cores overlap uploads of later ones end to end.

Device pipeline per core:
  - int8 [128, C] token tiles: dequant (DVE per-partition scale) to
    bf16, PE-transpose into feature-major kvT / qT
  - kT (feature-major), v (token-major), qhT projections via PE
  - per 128-query tile, per 2-head group: S matmuls into PSUM;
    multiplicative band mask (DVE) on exp(S); v carries a ones column
    so P^T @ v also yields softmax row-sums; 1/rowsum applied per head
    during the x PSUM->SBUF copy; PE-transpose x; output projection +
    bias; int8 row-quantize; DMA out.
"""

import numpy as np
import ml_dtypes

B, N, C, H = 4, 2048, 512, 8
HD = C // H  # 64
NCORES = 8
SEQ = N // 2  # rows per core
SCALE = HD ** -0.5
PB = 128
PWP = SEQ + PB  # padded kv rows per core
HG = 2          # heads per processing group


def _band_w(epoch: int):
    if epoch >= 60:
        return None
    if epoch < 22:
        return 4
    if epoch < 32:
        return 6
    if epoch < 42:
        return 8
    return 10


def _build_nc(w: int):
    import concourse.mybir as mybir
    import concourse.tile as tile
    from concourse import bacc
    from concourse.masks import make_identity

    f32 = mybir.dt.float32
    bf16 = mybir.dt.bfloat16
    i8 = mybir.dt.int8
    AF = mybir.ActivationFunctionType

    W2 = 2 * w
    WIN = PB + W2
    NQT = SEQ // PB
    CC = C // PB
    NVT = PWP // PB
    NG = H // HG

    KVROWS = SEQ + 2 * w      # uploaded kv rows per core (halo, no tile pad)
    T2 = KVROWS + SEQ         # total uploaded token rows per core
    NTILES = (T2 + PB - 1) // PB

    nc = bacc.Bacc(None, target_bir_lowering=False)
    # per-call activations: token-major int8 + per-token dequant scales;
    # rows [0:KVROWS] are kv (with halo), rows [KVROWS:] are q
    acts_d = nc.declare_dram_parameter("acts", [T2, C], i8, isOutput=False)
    ascale_d = nc.declare_dram_parameter("ascale", [PB, NTILES], f32, isOutput=False)
    # resident weights
    wkv_d = nc.declare_dram_parameter("wkv", [PB, CC * 2 * C], bf16, isOutput=False)
    wq_d = nc.declare_dram_parameter("wq", [PB, CC * C], bf16, isOutput=False)
    wp_d = nc.declare_dram_parameter("wp", [PB, CC * C], bf16, isOutput=False)
    bias_d = nc.declare_dram_parameter("bias_b", [PB, C], f32, isOutput=False)
    mask_d = nc.declare_dram_parameter(
        "mask", [PB, NQT * 2 * PB], bf16, isOutput=False
    )
    # flat int8 output: [SEQ*C] quantized rows, then the f32 per-query-row
    # scales bitcast to int8 as [PB, NQT*4]
    out_d = nc.declare_dram_parameter(
        "out", [SEQ * C + PB * NQT * 4], i8, isOutput=True
    )

    with tile.TileContext(nc) as tc:
        with (
            tc.sbuf_pool(name="const", bufs=1) as cpool,
            tc.sbuf_pool(name="work", bufs=3) as wpool,
            tc.psum_pool(name="psum", bufs=1) as ppool,
        ):
            # ---- persistent SBUF ----
            wq_s = cpool.tile([PB, CC, C], bf16)
            nc.sync.dma_start(wq_s, wq_d[:, :])
            wkv_s = cpool.tile([PB, CC, 2 * C], bf16)
            nc.sync.dma_start(wkv_s, wkv_d[:, :])
            wp_s = cpool.tile([PB, CC, C], bf16)
            nc.sync.dma_start(wp_s, wp_d[:, :])
            bias_s = cpool.tile([PB, C], f32)
            nc.sync.dma_start(bias_s, bias_d[:, :])
            mask_s = cpool.tile([PB, NQT, 2 * PB], bf16)
            nc.sync.dma_start(mask_s, mask_d[:, :])
            ident = cpool.tile([PB, PB], bf16)
            make_identity(nc, ident)

            # ---- dequant + on-device transpose into feature-major ----
            acts_s = cpool.tile([PB, NTILES, C], i8)
            for t in range(NTILES):
                lo, hi = t * PB, min((t + 1) * PB, T2)
                nc.sync.dma_start(acts_s[0 : hi - lo, t, :], acts_d[lo:hi, :])
            ascale_s = cpool.tile([PB, NTILES], f32)
            nc.sync.dma_start(ascale_s, ascale_d[:, :])

            kvT = cpool.tile([PB, CC, PWP], bf16)
            # key columns beyond the halo are never valid (mask gates them)
            # but must be finite zeros for the projections
            nc.vector.memset(kvT[:, :, KVROWS:PWP], 0.0)
            qT = cpool.tile([PB, CC, SEQ], bf16)
            for t in range(NTILES):
                lo, hi = t * PB, min((t + 1) * PB, T2)
                deq = wpool.tile([PB, C], bf16, tag="deq", bufs=2)
                nc.vector.tensor_scalar_mul(
                    deq, acts_s[:, t, :], ascale_s[:, t : t + 1]
                )
                trp = ppool.tile([PB, C], bf16, tag="big", bufs=2)
                for cc in range(CC):
                    nc.tensor.transpose(
                        trp[:, cc * PB : (cc + 1) * PB],
                        deq[:, cc * PB : (cc + 1) * PB],
                        ident,
                    )
                src3 = trp.rearrange("p (c q) -> p c q", c=CC)
                # route transposed columns: kv rows -> kvT, q rows -> qT
                segs = []
                if lo < KVROWS:
                    ln = min(hi, KVROWS) - lo
                    segs.append((kvT, lo, 0, ln))
                    if hi > KVROWS:
                        segs.append((qT, 0, ln, hi - KVROWS))
                else:
                    segs.append((qT, lo - KVROWS, 0, hi - lo))
                for dstbuf, dcol, scol, ln in segs:
                    nc.any.tensor_copy(
                        dstbuf[:, :, dcol : dcol + ln],
                        src3[:, :, scol : scol + ln],
                    )

            kT = cpool.tile([PB, CC, PWP], bf16)
            qhT = cpool.tile([PB, CC, SEQ], bf16)
            # v with an appended ones column per head: mm2 then yields
            # softmax row-sums for free in output column HD
            v_s = cpool.tile([PB, NVT, H, HD + 1], bf16)
            nc.vector.memset(v_s[:, :, :, HD], 1.0)

            def proj_T(dst, src, wsb, wofs, seqlen):
                segs = []
                s0 = 0
                while s0 < seqlen:
                    segs.append((s0, min(512, seqlen - s0)))
                    s0 += 512
                for co in range(CC):
                    for s0, sl in segs:
                        ps = ppool.tile([PB, 512], f32, tag="big", bufs=2)
                        for ci in range(CC):
                            nc.tensor.matmul(
                                ps[:, :sl],
                                wsb[:, ci, wofs + co * PB : wofs + (co + 1) * PB],
                                src[:, ci, s0 : s0 + sl],
                                start=(ci == 0),
                                stop=(ci == CC - 1),
                            )
                        nc.any.tensor_copy(dst[:, co, s0 : s0 + sl], ps[:, :sl])

            proj_T(qhT, qT, wq_s, 0, SEQ)
            proj_T(kT, kvT, wkv_s, 0, PWP)
            for i in range(NVT):
                ps = ppool.tile([PB, C], f32, tag="big", bufs=2)
                for ci in range(CC):
                    nc.tensor.matmul(
                        ps,
                        kvT[:, ci, i * PB : (i + 1) * PB],
                        wkv_s[:, ci, C : 2 * C],
                        start=(ci == 0),
                        stop=(ci == CC - 1),
                    )
                nc.any.tensor_copy(
                    v_s[:, i, :, :HD],
                    ps.rearrange("p (h d) -> p h d", d=HD),
                )

            # ---- attention + output projection per 128-query tile ----
            oscale_sb = cpool.tile([PB, NQT], f32)
            HH = H // 2  # heads per x psum half
            for t in range(NQT):
                x_half = [
                    ppool.tile([PB, HH, HD + 1], f32, tag="x", bufs=2, name=f"xh{t}_{i}")
                    for i in range(2)
                ]
                rinv = wpool.tile([PB, H], f32, tag="rinv", bufs=2)
                x_sb = wpool.tile([PB, C], bf16, tag="x_sb", bufs=2)
                for g in range(NG):
                    for hh in range(HG):
                        h = g * HG + hh
                        hc, hp = h // 2, (h % 2) * HD
                        # S^T against key tiles t and t+1 (band always fits):
                        # [key, chunk*query] layout, so P^T feeds mm2 directly
                        st = ppool.tile(
                            [PB, 256], f32, tag="s", bufs=4, name=f"st{t}_{h}"
                        )
                        for c in range(2):
                            nc.tensor.matmul(
                                st[:, c * PB : (c + 1) * PB],
                                kT[
                                    hp : hp + HD,
                                    hc,
                                    (t + c) * PB : (t + c + 1) * PB,
                                ],
                                qhT[hp : hp + HD, hc, t * PB : (t + 1) * PB],
                                start=True,
                                stop=True,
                            )
                        est = wpool.tile([PB, 256], bf16, tag="est", bufs=4)
                        nc.scalar.activation(est, st, AF.Exp, scale=SCALE)
                        nc.vector.tensor_mul(est, est, mask_s[:, t, :])
                        xp = x_half[h // HH]
                        for c in range(2):
                            nc.tensor.matmul(
                                xp[:, h % HH, :],
                                est[:, c * PB : (c + 1) * PB],
                                v_s[:, t + c, h, :],
                                start=(c == 0),
                                stop=(c == 1),
                            )
                    if (g * HG + HG) % HH == 0:
                        # heads for this x half done: 1/rowsum, normalize
                        half = (g * HG + HG) // HH - 1
                        xp = x_half[half]
                        nc.vector.reciprocal(
                            rinv[:, half * HH : (half + 1) * HH],
                            xp[:, :, HD],
                        )
                        for hh2 in range(HH):
                            h2 = half * HH + hh2
                            dst = x_sb[:, h2 * HD : (h2 + 1) * HD]
                            if hh2 % 2 == 0:
                                nc.vector.tensor_scalar_mul(
                                    dst, xp[:, hh2, :HD], rinv[:, h2 : h2 + 1]
                                )
                            else:
                                nc.scalar.activation(
                                    dst,
                                    xp[:, hh2, :HD],
                                    AF.Copy,
                                    scale=rinv[:, h2 : h2 + 1],
                                )
                xt_ps = ppool.tile([PB, C], bf16, tag="big", bufs=2)
                for ccI in range(CC):
                    nc.tensor.transpose(
                        xt_ps[:, ccI * PB : (ccI + 1) * PB],
                        x_sb[:, ccI * PB : (ccI + 1) * PB],
                        ident,
                    )
                xt_sb = wpool.tile([PB, C], bf16, tag="xt_sb")
                nc.any.tensor_copy(xt_sb, xt_ps)
                o_ps = ppool.tile([PB, C], f32, tag="big", bufs=2)
                for ci in range(CC):
                    nc.tensor.matmul(
                        o_ps,
                        xt_sb[:, ci * PB : (ci + 1) * PB],
                        wp_s[:, ci, :],
                        start=(ci == 0),
                        stop=(ci == CC - 1),
                    )
                out_sb = wpool.tile([PB, C], f32, tag="out_sb")
                nc.vector.tensor_add(out_sb, o_ps, bias_s)
                # int8 row-quantization: scale = rowmax/127
                absb = wpool.tile([PB, C], f32, tag="absb", bufs=2)
                nc.scalar.activation(absb, out_sb, AF.Abs)
                mx8 = wpool.tile([PB, 8], f32, tag="mx8", bufs=2)
                nc.vector.max(mx8, absb)
                rin = wpool.tile([PB, 2], f32, tag="rin", bufs=2)
                nc.vector.reciprocal(rin[:, 0:1], mx8[:, 0:1])
                nc.vector.tensor_scalar_mul(rin[:, 1:2], rin[:, 0:1], 127.0)
                oq = wpool.tile([PB, C], i8, tag="oq", bufs=2)
                nc.scalar.activation(oq, out_sb, AF.Copy, scale=rin[:, 1:2])
                nc.sync.dma_start(
                    out_d[t * PB * C : (t + 1) * PB * C].rearrange(
                        "(p c) -> p c", c=C
                    ),
                    oq,
                )
                nc.vector.tensor_scalar_mul(
                    oscale_sb[:, t : t + 1], mx8[:, 0:1], 1.0 / 127.0
                )
            nc.sync.dma_start(
                out_d[SEQ * C : SEQ * C + PB * NQT * 4].rearrange(
                    "(p j) -> p j", j=NQT * 4
                ),
                oscale_sb.bitcast(i8),
            )

    nc.compile()
    return nc


_NC_CACHE = {}
_RUNNER_CACHE = {}
LAST_RESULTS = None


def _get_nc(w: int):
    if w not in _NC_CACHE:
        _NC_CACHE[w] = _build_nc(w)
    return _NC_CACHE[w]


def _numpy_reference(kv, q, Wkv, Wq, Wproj, bproj, epoch):
    # dense fallback (epoch >= 60)
    b, n, c = kv.shape
    hd = c // H
    kvp = (kv @ Wkv).reshape(b, n, 2, H, hd)
    k = kvp[:, :, 0].transpose(0, 2, 1, 3)
    v = kvp[:, :, 1].transpose(0, 2, 1, 3)
    qh = (q @ Wq).reshape(b, n, H, hd).transpose(0, 2, 1, 3)
    attn = np.einsum("bhnd,bhmd->bhnm", qh, k) * (hd ** -0.5)
    w = _band_w(int(epoch))
    if w is not None:
        idx = np.arange(n)
        mask = np.abs(idx[:, None] - idx[None, :]) <= w
        attn = np.where(mask[None, None], attn, np.float32(-1e9))
    attn = attn - attn.max(axis=-1, keepdims=True)
    attn = np.exp(attn)
    attn /= attn.sum(axis=-1, keepdims=True)
    x = np.einsum("bhnm,bhmd->bhnd", attn, v)
    x = x.transpose(0, 2, 1, 3).reshape(b, n, c)
    return (x @ Wproj + bproj).astype(np.float32)


def _chunkW(wmat):
    """[C, M] -> [128, CC*M]: out[p, cc*M+m] = w[cc*128+p, m]"""
    M = wmat.shape[1]
    return np.ascontiguousarray(
        wmat.reshape(-1, PB, M).transpose(1, 0, 2).reshape(PB, -1)
    )


def _pack_weights(Wkv, Wq, Wproj, bproj, w):
    """Weight-class tensors: uploaded once, kept device-resident."""
    bf = ml_dtypes.bfloat16
    W2, NQT = 2 * w, SEQ // PB
    shared = {
        "wkv": _chunkW(Wkv).astype(bf),
        "wq": _chunkW(Wq).astype(bf),
        "wp": _chunkW(Wproj).astype(bf),
        "bias_b": np.broadcast_to(bproj, (PB, C)).copy().astype(np.float32),
    }

    # band mask in S^T-chunk coords [t, k, c, q]
    t_idx = np.arange(NQT)[:, None, None, None]
    k_idx = np.arange(PB)[None, :, None, None]
    c_idx = np.arange(2)[None, None, :, None]
    q_idx = np.arange(PB)[None, None, None, :]
    band2 = (q_idx <= c_idx * PB + k_idx) & (c_idx * PB + k_idx <= q_idx + W2)

    masks = []
    for core in range(NCORES):
        b, half = divmod(core, 2)
        r0 = half * SEQ
        # S^T chunk mask: entry [k, t, c*128+q] gates key 128(t+c)+k
        # (padded coords) against query 128t+q
        kg = r0 + (t_idx + c_idx) * PB + k_idx - w
        valid = band2 & (kg >= 0) & (kg < N)
        masks.append(valid.astype(np.float32).transpose(1, 0, 2, 3).reshape(PB, -1))
    sharded = {"mask": np.ascontiguousarray(np.concatenate(masks, 0)).astype(bf)}
    return shared, sharded


def _quant_into(a, dst, tmp):
    """int8-quantize [R, C] rows with per-row scales directly into dst."""
    s = np.abs(a).max(axis=1)
    r = 127.0 / np.maximum(s, 1e-30)
    t = tmp[: a.shape[0]]
    np.multiply(a, r[:, None], out=t)
    np.rint(t, out=t)
    np.copyto(dst, t, casting="unsafe")
    return s * (1.0 / 127.0)


def _pack_and_put(kv, q, w, runner):
    """Quantize + pack per-core [T2, C] int8 shards, streaming each to its
    device as soon as it's built so transfers overlap packing."""
    import jax

    KVROWS = SEQ + 2 * w
    T2 = KVROWS + SEQ
    NTILES = (T2 + PB - 1) // PB
    tmp = np.empty((KVROWS, C), np.float32)
    arrs = []
    scales_all = np.full((NCORES, NTILES * PB), 1.0 / 127.0, np.float32)
    for core in range(NCORES):
        b, half = divmod(core, 2)
        r0 = half * SEQ
        lo, hi = max(0, r0 - w), min(N, r0 + SEQ + w)
        d0 = lo - (r0 - w)
        shard = np.zeros((T2, C), np.int8)
        scales_all[core, d0 : d0 + hi - lo] = _quant_into(
            kv[b, lo:hi], shard[d0 : d0 + hi - lo], tmp
        )
        scales_all[core, KVROWS : KVROWS + SEQ] = _quant_into(
            q[b, r0 : r0 + SEQ], shard[KVROWS:], tmp
        )
        arrs.append(jax.device_put(shard, runner.devices[core]))
    acts = jax.make_array_from_single_device_arrays(
        (NCORES * T2, C), runner.sh_core, arrs
    )
    ascale = np.ascontiguousarray(
        scales_all.reshape(NCORES, NTILES, PB).transpose(0, 2, 1)
    ).reshape(NCORES * PB, NTILES)
    return acts, ascale


class _Runner:
    """Cached jit(shard_map) over the Bass NEFF with device-resident weights
    and donated-output recycling (mirrors bass2jax.run_bass_via_pjrt)."""

    PER_CALL = ("acts", "ascale")
    SHARDED_WEIGHTS = ("mask",)

    def __init__(self, nc):
        import jax
        import concourse.mybir as mybir
        from concourse import bass2jax
        from jax.sharding import Mesh, PartitionSpec, NamedSharding

        bass2jax.install_neuronx_cc_hook()
        try:
            from jax.experimental.shard_map import shard_map
        except ImportError:
            import functools

            shard_map = functools.partial(jax.shard_map)

        self.jax = jax
        self.nc = nc
        devices = jax.devices()[:NCORES]
        assert len(devices) == NCORES
        self.devices = devices
        self.mesh = Mesh(np.asarray(devices), ("core",))
        P = PartitionSpec
        self.sh_core = NamedSharding(self.mesh, P("core"))
        self.sh_repl = NamedSharding(self.mesh, P())

        partition_name = (
            nc.partition_id_tensor.name if nc.partition_id_tensor else None
        )
        in_names, out_names, out_avals, zero_outs = [], [], [], []
        in_specs = []
        for alloc in nc.m.functions[0].allocations:
            if not isinstance(alloc, mybir.MemoryLocationSet):
                continue
            name = alloc.memorylocations[0].name
            if alloc.kind == "ExternalInput":
                if name != partition_name:
                    in_names.append(name)
                    in_specs.append(
                        P("core")
                        if (name in self.PER_CALL or name in self.SHARDED_WEIGHTS)
                        else P()
                    )
            elif alloc.kind == "ExternalOutput":
                assert alloc.tensor_shape is not None and alloc.dtype is not None
                out_names.append(name)
                shape = tuple(alloc.tensor_shape)
                dtype = mybir.dt.np(alloc.dtype)
                out_avals.append(jax.core.ShapedArray(shape, dtype))
                zero_outs.append(
                    np.zeros((NCORES * shape[0], *shape[1:]), dtype)
                )
        self.in_names = in_names
        self.out_names = out_names
        n_params = len(in_names)
        all_in_names = tuple(in_names) + tuple(out_names)
        if partition_name is not None:
            all_in_names = all_in_names + (partition_name,)
        donate = tuple(range(n_params, n_params + len(out_names)))
        in_specs = tuple(in_specs) + (P("core"),) * len(out_names)
        out_specs = (P("core"),) * len(out_names)
        out_avals_t = tuple(out_avals)

        def _body(*args):
            operands = list(args)
            if partition_name is not None:
                operands.append(bass2jax.partition_id_tensor())
            outs = bass2jax._bass_exec_p.bind(
                *operands,
                out_avals=out_avals_t,
                in_names=all_in_names,
                out_names=tuple(out_names),
                lowering_input_output_aliases=(),
                sim_require_finite=True,
                sim_require_nnan=True,
                nc=nc,
            )
            return tuple(outs)

        self.fn = jax.jit(
            shard_map(
                _body,
                mesh=self.mesh,
                in_specs=in_specs,
                out_specs=out_specs,
                check_rep=False,
            ),
            donate_argnums=donate,
            keep_unused=True,
        )
        # donated output buffers, recycled call-to-call (kernel writes every
        # element of out, so stale contents are harmless)
        self.zero_outs = zero_outs
        self.recycle = [jax.device_put(z, self.sh_core) for z in zero_outs]
        self.weights_raw = None
        self.weights_dev = None
        from concurrent.futures import ThreadPoolExecutor

        self.fetch_pool = ThreadPoolExecutor(NCORES)

    def ensure_weights(self, Wkv, Wq, Wproj, bproj, w):
        raw = (Wkv, Wq, Wproj, bproj)
        if self.weights_raw is not None and all(
            a is b or np.array_equal(a, b)
            for a, b in zip(self.weights_raw, raw)
        ):
            return
        shared, sharded = _pack_weights(Wkv, Wq, Wproj, bproj, w)
        dev = {}
        for name, arr in shared.items():
            dev[name] = self.jax.device_put(arr, self.sh_repl)
        for name, arr in sharded.items():
            dev[name] = self.jax.device_put(arr, self.sh_core)
        for v in dev.values():
            v.block_until_ready()
        self.weights_dev = dev
        self.weights_raw = tuple(np.array(a, copy=True) for a in raw)

    def run(self, per_call):
        """Dispatch; returns the on-device output arrays (not fetched)."""
        args = [
            per_call[name] if name in self.PER_CALL else self.weights_dev[name]
            for name in self.in_names
        ]
        try:
            outs = self.fn(*args, *self.recycle)
        except Exception:
            # the donated recycle buffers may have been consumed; restore
            # fresh ones so a retry by the caller can still work
            self.recycle = [
                self.jax.device_put(z, self.sh_core) for z in self.zero_outs
            ]
            raise
        self.recycle = list(outs)
        return outs


def _get_runner(w: int):
    if w not in _RUNNER_CACHE:
        _RUNNER_CACHE[w] = _Runner(_get_nc(w))
    return _RUNNER_CACHE[w]


def kernel(**inputs):
    kv = np.asarray(inputs["kv"], np.float32)
    q = np.asarray(inputs["q"], np.float32)
    Wkv = np.asarray(inputs["Wkv"], np.float32)
    Wq = np.asarray(inputs["Wq"], np.float32)
    Wproj = np.asarray(inputs["Wproj"], np.float32)
    bproj = np.asarray(inputs["bproj"], np.float32)
    epoch = int(np.asarray(inputs["epoch"]))

    w = _band_w(epoch)
    if w is None:
        return _numpy_reference(kv, q, Wkv, Wq, Wproj, bproj, epoch)

    runner = _get_runner(w)
    runner.ensure_weights(Wkv, Wq, Wproj, bproj, w)
    acts, ascale = _pack_and_put(kv, q, w, runner)
    outs = runner.run({"acts": acts, "ascale": ascale})

    # fetch shards concurrently (round trips pipeline only when issued in
    # parallel), dequantizing each on the main thread as it completes
    from concurrent.futures import as_completed

    NQT = SEQ // PB
    PER = SEQ * C + PB * NQT * 4
    out = np.empty((NCORES, SEQ, C), np.float32)
    futs = {
        runner.fetch_pool.submit(lambda s=s: np.asarray(s.data)): s.index[
            0
        ].start
        // PER
        for s in outs[0].addressable_shards
    }
    for f in as_completed(futs):
        core = futs[f]
        arr = f.result()
        rows = arr[SEQ * C :].view(np.float32).reshape(PB, NQT).T.ravel()
        np.multiply(
            arr[: SEQ * C].reshape(SEQ, C), rows[:, None], out=out[core]
        )
    return out.reshape(B, N, C)

